# revision 37
# baseline (speedup 1.0000x reference)
"""Trainium2 Bass kernel for attention pooling (nn_AttentionLayer).

Reference math (per batch b):
    score  = tanh(x @ W + b)        # [S, D]
    logits = score @ V              # [S, 1]
    attn   = softmax(logits, axis=S)
    out    = sum_s attn[s] * x[s]   # [D]

Sharding: data-parallel over batch across 8 NeuronCores (4 batches/core).
W/b/V replicated. No collectives.

Per-core dataflow (B_LOC=4, S=4096, D=256; seq chunks of SC=2048, folded
s = s0 + p*16 + f so each partition's source rows stay contiguous):
  1. SWDGE cast-DMA HBM->SBUF f32->bf16 into x_nat[p, f, d], issued as
     fold-half loads (quarters for chunk 0) and WAR-paced by the x pool
     depth - the DGE completion-sem rings are only 8 deep per class, so
     DMA issue must track consumption or recycling fences serialize
     everything.
  2. xT[d_low, (f,dc), s_low] built per 4-fold block, split between the
     DMA xbar (half-chunk transposes chained right behind their feeding
     load) and the PE (is_transpose matmuls into one PSUM bank, DVE
     copy-out), balancing the DMA and PE devices.
  3. scoreT[e, s] = W.T @ x.T on TensorE (W stationary, xT moving),
     PSUM pair tiles [128, 2(ec), 512].
  4. one ACT tanh per pair -> st bf16 (b==0 per the problem spec, so both
     ec halves merge into one instruction; a safe per-ec-bias build is
     compiled on demand if b is ever nonzero).
  5. logits via fat-stationary/skinny-moving matmuls: stationary =
     st[:, ec, 128-col block], moving = V chunk [128, 1] -> one PSUM
     column accumulated over ec. Lands logitsT directly in natural
     layout (PL region of the shared MISC psum bank) - no collect/
     scatter/transpose chain, and ldweights/1-col matmuls are ~free.
  6. one ACT exp per batch (PSUM -> elog bf16, accum_out -> denominator
     partials per partition; host sums the 128 partials).
  7. numerator with the same trick: stationary = x_nat[:, f, dc*128:...],
     moving = elog column [128, 1], accumulated into MISC NUM columns.
  8. outputs merge into one [128, 16] tensor; the bulk streams out while
     the last chunk's numerator still runs. Host does the divide.
The last batch's exp/numerator is split per chunk so half overlaps the
final score groups, shrinking the serial tail.
"""

import os
import sys

import numpy as np

_TRN_REPO = "/opt/trn_rl_repo"

B, S, D = 32, 4096, 256
N_CORES = 8
B_LOC = B // N_CORES          # 4 batches per core
SC = 2048                     # seq chunk
F = SC // 128                 # folds per chunk (16); s = s0 + p*F + f
NCH = S // SC                 # chunks per batch (2)
NCHT = B_LOC * NCH            # chunks per core (8)
NSLOT = 4 * NCHT              # score groups (512 seqs) per core (32)

# transpose blocks (chunk, group 0..3) handled on the PE instead of the
# DMA xbar; chunk 0/1 cover the pipeline fill, the late chunks offload
# the DMA device near the end
PE_BLOCKS = {(ci, g) for ci in range(NCHT) for g in range(4)}
N_SPIN = 22                   # PE warm-up spin matmuls (128 cols each)

_cache = {}


def _build(use_bias=False):
    sys.path.insert(0, _TRN_REPO)
    import concourse.bacc as bacc
    import concourse.tile as tile
    from concourse import mybir

    f32 = mybir.dt.float32
    bf16 = mybir.dt.bfloat16

    nc = bacc.Bacc("TRN2", target_bir_lowering=False, debug=False)

    x_d = nc.dram_tensor("inputs", (B_LOC, S, D), f32, kind="ExternalInput")
    W_d = nc.dram_tensor("W", (D, D), f32, kind="ExternalInput")
    b_d = nc.dram_tensor("b", (D,), f32, kind="ExternalInput")
    V_d = nc.dram_tensor("V", (D, 1), f32, kind="ExternalInput")
    # numerators: cols 0-5 num b0..b2, 6-7 chunk6, 8-9 chunk7 folds 0-8,
    # 10-11 chunk7 folds 8-16.  Denominator partial rows live in a PSUM row
    # (cols 144.. of the MISC bank) and stream out separately.
    out_d = nc.dram_tensor("out", (128, 12), f32, kind="ExternalOutput")
    den_d = nc.dram_tensor("den", (1, 128), f32, kind="ExternalOutput")

    with tile.TileContext(nc) as tc:
        with (
            tc.tile_pool(name="consts", bufs=1) as consts,
            tc.tile_pool(name="xpool", bufs=6) as xpool,
            tc.tile_pool(name="xtpool", bufs=4) as xtpool,
            tc.tile_pool(name="stpool", bufs=5) as stpool,
            tc.tile_pool(name="smalls", bufs=1) as smalls,
            tc.tile_pool(name="pspool", bufs=2, space="PSUM") as pspool,
            tc.tile_pool(name="trpool", bufs=3, space="PSUM") as trpool,
            tc.tile_pool(name="miscpool", bufs=1, space="PSUM") as miscpool,
        ):
            # ---- dependency-free prologue first: DVE memsets + Pool
            #      identity build, so PE transposes / warm-up spins aren't
            #      stuck behind loads or casts in those queues ----
            ones_sb = consts.tile([128, 128], bf16)
            nc.vector.memset(ones_sb, 1.0)
            dummy_sb = consts.tile([128, 2], bf16)
            nc.vector.memset(dummy_sb, 0.0)
            dummy_mov = consts.tile([128, 128], bf16)
            nc.vector.memset(dummy_mov, 0.0)
            zero_bias = consts.tile([128, 1], f32)
            nc.vector.memset(zero_bias, 0.0)
            I_sb = consts.tile([128, 128], bf16)
            nc.gpsimd.affine_select(
                out=I_sb,
                in_=ones_sb,
                pattern=[[-1, 128]],
                compare_op=mybir.AluOpType.is_equal,
                fill=0.0,
                base=0,
                channel_multiplier=1,
            )

            # params on HWDGE/SP; small ones first so the big W transfer
            # doesn't delay the first x load on the shared DMA device
            V_f = consts.tile([128, 2], f32)
            nc.sync.dma_start(
                out=V_f, in_=V_d[:, :].rearrange("(ec p) o -> p (ec o)", p=128)
            )
            b_sb = consts.tile([128, 2], f32)
            nc.sync.dma_start(
                out=b_sb, in_=b_d[:].rearrange("(ec p) -> p ec", p=128)
            )
            W_f = consts.tile([128, 2, D], f32)
            nc.sync.dma_start(
                out=W_f, in_=W_d[:, :].rearrange("(dc p) e -> p dc e", p=128)
            )
            W_sb = consts.tile([128, 2, D], bf16)
            nc.vector.tensor_copy(out=W_sb, in_=W_f)
            V_sb = consts.tile([128, 2], bf16)
            nc.vector.tensor_copy(out=V_sb, in_=V_f)

            # pre-create x/xT tiles in usage order: pool slots rotate as a
            # strict ring in creation order, so creation order must match
            # consumption order or an early chunk's tile can end up waiting
            # on a later chunk's death (deadlock / serialization)
            x_nat = {}
            xT = {}
            for _ci in range(NCHT):
                x_nat[_ci] = xpool.tile([128, F, D], bf16, name="x_nat", tag="x")
                xT[_ci] = xtpool.tile(
                    [128, 2 * F, 128], bf16, name="xT", tag="xT"
                )

            def load_part(ci, f0, f1):
                # fold range [f0, f1) of chunk ci; per-partition source rows
                # stay contiguous (s = s0 + p*F + f)
                bb, c = divmod(ci, NCH)
                s0 = c * SC
                src = x_d[bb, s0 : s0 + SC, :].rearrange("(p f) d -> p f d", p=128)
                nc.gpsimd.dma_start(
                    out=x_nat[ci][:, f0:f1, :], in_=src[:, f0:f1, :]
                )

            def dma_transpose_half(ci, h):
                # folds [8h, 8h+8) -> xT rows [16h, 16h+16)
                nc.sync.dma_start(
                    out=xT[ci][:, 16 * h : 16 * h + 16, :],
                    in_=x_nat[ci][:, 8 * h : 8 * h + 8, :],
                    transpose=True,
                )

            def pe_transpose_block(ci, g):
                # folds [4g, 4g+4) -> 8 [128,128] tiles -> 1 psum bank
                tr = trpool.tile([128, 1024], bf16, name="tr", tag="tr")
                for t in range(8):
                    f = 4 * g + t // 2
                    dc = t % 2
                    nc.tensor.transpose(
                        out=tr[:, t * 128 : (t + 1) * 128],
                        in_=x_nat[ci][:, f, dc * 128 : (dc + 1) * 128],
                        identity=I_sb,
                    )
                nc.vector.tensor_copy(
                    out=xT[ci][:, 8 * g : 8 * g + 8, :],
                    in_=tr.rearrange("p (a b) -> p a b", a=8),
                )

            def issue_chunk_dmas(ci):
                # fold-half loads (quarters for chunk 0), each DMA-xbar
                # half-transpose chained right behind the load that feeds it
                # so the shared DMA device serves them in need-order
                if ci == 0:
                    parts = [(0, 4), (4, 8), (8, 12), (12, 16)]
                else:
                    parts = [(0, 8), (8, 16)]
                for f0, f1 in parts:
                    load_part(ci, f0, f1)
                    if f1 % 8 == 0:
                        h = f1 // 8 - 1
                        if (ci, 2 * h) not in PE_BLOCKS:
                            dma_transpose_half(ci, h)

            # shared psum bank: PL logits cols 0..127, NUM cols 128..141,
            # DEN partial rows [0:1, 144:272], spin target cols 384..511
            MISC = miscpool.tile([128, 512], f32, name="MISC")

            def spin(n):
                for _ in range(n):
                    nc.tensor.matmul(
                        MISC[0:2, 384:512],
                        dummy_sb,
                        dummy_mov,
                        start=True,
                        stop=True,
                    )

            # chunks 0-2 issued up-front; the rest in the slot loop
            # (usage order, WAR-paced by the pools)
            issue_chunk_dmas(0)
            issue_chunk_dmas(1)
            issue_chunk_dmas(2)

            spin(N_SPIN)

            # ---- outputs / softmax state ----
            out_sb = smalls.tile([128, 12], f32, name="out_sb")
            den_sb = smalls.tile([1, 128], f32, name="den_sb")
            elog = {}
            st_tiles = {}

            def score_group(q):
                ci, g = divmod(q, 4)
                ps = pspool.tile([128, 2, 512], f32, name="ps", tag="ps")
                xv = xT[ci].rearrange("p (f dc) s -> p f dc s", dc=2)
                for ec in range(2):
                    for dc in range(2):
                        nc.tensor.matmul(
                            ps[:, ec, :],
                            W_sb[:, dc, ec * 128 : (ec + 1) * 128],
                            xv[:, 4 * g : 4 * g + 4, dc, :],
                            start=(dc == 0),
                            stop=(dc == 1),
                        )
                st = stpool.tile([128, 2, 512], bf16, name="st", tag="st")
                if use_bias:
                    for ec in range(2):
                        nc.scalar.activation(
                            out=st[:, ec, :],
                            in_=ps[:, ec, :],
                            func=mybir.ActivationFunctionType.Tanh,
                            bias=b_sb[:, ec : ec + 1],
                            scale=1.0,
                        )
                else:
                    nc.scalar.activation(
                        out=st,
                        in_=ps,
                        func=mybir.ActivationFunctionType.Tanh,
                        bias=zero_bias[:, 0:1],
                        scale=1.0,
                    )
                st_tiles[q] = st

            def logits_group(q):
                ci, g = divmod(q, 4)
                bb, c = divmod(ci, NCH)
                st = st_tiles.pop(q)
                for k in range(4):
                    col = bb * 32 + c * 16 + g * 4 + k
                    for ec in range(2):
                        nc.tensor.matmul(
                            MISC[:, col : col + 1],
                            st[:, ec, k * 128 : (k + 1) * 128],
                            V_sb[:, ec : ec + 1],
                            start=(ec == 0),
                            stop=(ec == 1),
                        )

            def exp_piece(bb, c0, c1, den_col):
                # exp over elog cols [c0, c1) of batch bb; denominator
                # partials land in the PSUM den row via a ones-stationary
                # matmul (cheaper than ACT's accumulator read-out)
                if bb not in elog:
                    elog[bb] = smalls.tile(
                        [128, 32], bf16, name="elog", tag="elog", bufs=2
                    )
                nc.scalar.activation(
                    out=elog[bb][:, c0:c1],
                    in_=MISC[:, bb * 32 + c0 : bb * 32 + c1],
                    func=mybir.ActivationFunctionType.Exp,
                )
                nc.tensor.matmul(
                    MISC[0:1, den_col : den_col + (c1 - c0)],
                    ones_sb[:, 0:1],
                    elog[bb][:, c0:c1],
                    start=True,
                    stop=True,
                )

            def num_block(bb, pieces, numcol0):
                # accumulate numerator over (chunk, f0, f1) pieces into
                # MISC cols numcol0 (dc=0) / numcol0+1 (dc=1)
                for dc in range(2):
                    first = True
                    for pi, (ci, f0, f1) in enumerate(pieces):
                        c = ci % NCH
                        last_piece = pi == len(pieces) - 1
                        for f in range(f0, f1):
                            nc.tensor.matmul(
                                MISC[:, numcol0 + dc : numcol0 + dc + 1],
                                x_nat[ci][:, f, dc * 128 : (dc + 1) * 128],
                                elog[bb][:, c * 16 + f : c * 16 + f + 1],
                                start=first,
                                stop=(last_piece and f == f1 - 1),
                            )
                            first = False

            def num_copy(numcol0, outcol0, n=2):
                nc.vector.tensor_copy(
                    out=out_sb[:, outcol0 : outcol0 + n],
                    in_=MISC[:, numcol0 : numcol0 + n],
                )

            # ---- main software-pipelined slot loop ----
            pe_t_pending = sorted(PE_BLOCKS)

            for q in range(NSLOT):
                ci, g = divmod(q, 4)
                # keep loads ~3 chunks ahead (plus WAR pacing from xpool)
                if g == 0 and ci + 3 < NCHT:
                    issue_chunk_dmas(ci + 3)
                if q == 31:
                    # chunk 7a exp sits in the ACT queue BEFORE tanh(31)
                    # (emitted ahead of score_group(31)) so the only ACT
                    # work after the final tanh is the final exp piece
                    exp_piece(3, 16, 24, 256)
                # blocks this slot's score needs must precede it; prefetch
                # blocks go after the score so a late load stalls prefetch
                # work rather than the score itself
                while pe_t_pending and 4 * pe_t_pending[0][0] + pe_t_pending[0][1] <= q:
                    tci, tg = pe_t_pending.pop(0)
                    pe_transpose_block(tci, tg)
                score_group(q)
                if pe_t_pending:
                    tci, tg = pe_t_pending[0]
                    if 4 * tci + tg <= q + 5:
                        pe_t_pending.pop(0)
                        pe_transpose_block(tci, tg)
                if q >= 2 and q <= 28:
                    logits_group(q - 2)
                    qq = q - 2
                    if qq % 8 == 7 and qq // 8 < 3:
                        exp_piece(qq // 8, 0, 32, 144 + 32 * (qq // 8))
                if q >= 4 and (q - 4) % 8 == 7:
                    bbq = (q - 4) // 8
                    if bbq < 3:
                        num_block(
                            bbq,
                            [(NCH * bbq, 0, F), (NCH * bbq + 1, 0, F)],
                            128 + 2 * bbq,
                        )
                        num_copy(128 + 2 * bbq, 2 * bbq)
                # shorter logits lag near the tail (PE has slack there)
                if q == 29:
                    logits_group(27)
                    logits_group(28)
                    exp_piece(3, 0, 16, 240)
                if q == 30:
                    logits_group(29)
                if q == 31:
                    logits_group(30)
                    num_block(3, [(6, 0, F)], 134)
                    num_copy(134, 6)
                    num_block(3, [(7, 0, 8)], 136)
                    num_copy(136, 8)
                    nc.sync.dma_start(out=out_d[:, 0:8], in_=out_sb[:, 0:8])
                    nc.sync.dma_start(out=out_d[:, 8:10], in_=out_sb[:, 8:10])

            # ---- tail: only the last chunk's second-half work remains ----
            logits_group(31)
            exp_piece(3, 24, 32, 264)
            num_block(3, [(7, 8, F)], 138)
            num_copy(138, 10)

            nc.vector.tensor_copy(out=den_sb, in_=MISC[0:1, 144:272])
            nc.sync.dma_start(out=out_d[:, 10:12], in_=out_sb[:, 10:12])
            nc.scalar.dma_start(out=den_d[:, :], in_=den_sb)

    nc.compile()
    return nc


def _get_nc(use_bias=False):
    key = "nc_bias" if use_bias else "nc"
    if key not in _cache:
        _cache[key] = _build(use_bias)
    return _cache[key]


def kernel(inputs, W, b, V):
    sys.path.insert(0, _TRN_REPO)
    from concourse.bass_utils import run_bass_kernel_spmd

    inputs = np.ascontiguousarray(np.asarray(inputs, dtype=np.float32))
    W = np.ascontiguousarray(np.asarray(W, dtype=np.float32))
    b = np.ascontiguousarray(np.asarray(b, dtype=np.float32))
    V = np.ascontiguousarray(np.asarray(V, dtype=np.float32))

    # the fast build fuses tanh across both e-chunks, which requires b == 0
    # (guaranteed by the problem spec); fall back to a per-ec-bias build if
    # a nonzero bias ever shows up
    use_bias = bool(np.any(b != 0.0))
    nc = _get_nc(use_bias)

    in_maps = [
        {
            "inputs": inputs[i * B_LOC : (i + 1) * B_LOC],
            "W": W,
            "b": b,
            "V": V,
        }
        for i in range(N_CORES)
    ]

    trace = bool(int(os.environ.get("BENCH_TRACE", "0")))
    try:
        res = run_bass_kernel_spmd(
            nc, in_maps, core_ids=list(range(N_CORES)), trace=trace
        )
    except ModuleNotFoundError:
        res = run_bass_kernel_spmd(
            nc, in_maps, core_ids=list(range(N_CORES)), trace=False
        )
    _cache["last_exec_time_ns"] = res.exec_time_ns
    _cache["last_result"] = res
    outs = []
    for r in res.results:
        o = r["out"]            # [128, 12] numerators
        dn = r["den"][0]        # [128] denominator partials (cols 144..272)
        ctx = np.empty((B_LOC, D), dtype=np.float32)
        for bb in range(3):
            den = dn[32 * bb : 32 * bb + 32].sum()
            ctx[bb, :128] = o[:, 2 * bb] / den
            ctx[bb, 128:] = o[:, 2 * bb + 1] / den
        den3 = dn[96:128].sum()
        ctx[3, :128] = (o[:, 6] + o[:, 8] + o[:, 10]) / den3
        ctx[3, 128:] = (o[:, 7] + o[:, 9] + o[:, 11]) / den3
        outs.append(ctx)
    return np.concatenate(outs, axis=0)
